# revision 1
# baseline (speedup 1.0000x reference)
"""Trainium2 Bass kernel for nn_ModelWithLoss_67808943669610.

Computes, for the full (unsharded) inputs:
    logits = x @ W + b                       # [B, C]
    total  = sum_c exp(logits)               # per row
    pos    = logits gathered at labels       # [B, K]
    loss   = mean over (B*K) of log(exp(pos) + total - sum_k exp(pos)) - pos

Sharding: data-parallel over the batch. Each of the 8 cores gets 128 rows of
x/labels and a full copy of W (bf16, laid out as two stacked 64-row halves so
DMA tiles span 128 partitions at full bandwidth). Per core:
  - PE streams W through two persistent K=64 weight blocks (xT duplicated in
    rows 0-63 / 64-127) producing logits in PSUM, 2048 classes per chunk.
  - ScalarE does exp with fused free-axis accumulation (accum_out), one
    [128, 2, nf] instruction per chunk, writing per-chunk partial sums.
  - Positive logits are computed separately: indirect-DMA gather of the 640
    needed W^T rows + a DVE dot against x, all in fp32.
  - Final per-core scalar = sum of per-(row,positive) losses / (B*K); the
    host just sums the 8 per-core scalars.
The max-subtraction in the reference cancels algebraically; logits here are
O(1) so unshifted exp is numerically safe in fp32.
"""

import numpy as np

B, D, C, KPOS = 1024, 64, 100000, 5
NCORES = 8
RPC = B // NCORES          # 128 rows per core
CHALF = C // 2             # 50000 classes per half-block
NF = 1024                  # classes per half-block per PSUM chunk
PSUM_W = 2 * NF            # psum tile free width (two half-blocks)
NCHUNKS = -(-CHALF // NF)  # 49


def _ensure_concourse():
    try:
        import concourse  # noqa: F401
    except ImportError:
        import sys
        for p in ("/opt/trn_rl_repo", "/root/.axon_site/_ro/trn_rl_repo"):
            if p not in sys.path:
                sys.path.insert(0, p)


def build_program(has_bias: bool):
    _ensure_concourse()
    import concourse.bass as bass
    import concourse.bacc as bacc
    import concourse.mybir as mybir
    import concourse.tile as tile

    f32 = mybir.dt.float32
    bf16 = mybir.dt.bfloat16
    i32 = mybir.dt.int32
    AF = mybir.ActivationFunctionType
    ALU = mybir.AluOpType
    AX = mybir.AxisListType

    nc = bacc.Bacc(
        "TRN2",
        target_bir_lowering=False,
        debug=False,
        num_devices=NCORES,
    )

    w2 = nc.dram_tensor("w2", [128, CHALF], bf16, kind="ExternalInput")
    xt2 = nc.dram_tensor("xt2", [128, 128], bf16, kind="ExternalInput")
    wt = nc.dram_tensor("wt", [C, D], f32, kind="ExternalInput")
    bcol = nc.dram_tensor("bcol", [C, 1], f32, kind="ExternalInput")
    labels_d = nc.dram_tensor("labels", [RPC, KPOS], i32, kind="ExternalInput")
    xs_d = nc.dram_tensor("xs", [RPC, D], f32, kind="ExternalInput")
    if has_bias:
        b2_d = nc.dram_tensor("b2", [2, CHALF], f32, kind="ExternalInput")
    loss_d = nc.dram_tensor("loss", [1, 1], f32, kind="ExternalOutput")

    with tile.TileContext(nc) as tc:
        with (
            tc.tile_pool(name="wpool", bufs=4) as wpool,
            tc.tile_pool(name="psum", bufs=2, space="PSUM") as pp,
            tc.tile_pool(name="esp", bufs=1) as esp,
            tc.tile_pool(name="small", bufs=1) as sp,
        ):
            xt_sb = sp.tile([128, 128], bf16)
            nc.sync.dma_start(out=xt_sb[:], in_=xt2[:])
            acc = sp.tile([128, NCHUNKS], f32)
            es = esp.tile([128, PSUM_W], f32)

            if has_bias:
                ones33 = sp.tile([33, 128], f32)
                nc.vector.memset(ones33[:], 1.0)

            # ---- positives: gather the needed W^T rows, dot with x ----
            labels_sb = sp.tile([RPC, KPOS], i32)
            nc.sync.dma_start(out=labels_sb[:], in_=labels_d[:])
            xs_sb = sp.tile([RPC, D], f32)
            nc.sync.dma_start(out=xs_sb[:], in_=xs_d[:])
            gat = sp.tile([RPC, KPOS * D], f32)
            bg = sp.tile([RPC, KPOS], f32)
            for k in range(KPOS):
                nc.gpsimd.indirect_dma_start(
                    out=gat[:, k * D:(k + 1) * D],
                    out_offset=None,
                    in_=wt[:, :],
                    in_offset=bass.IndirectOffsetOnAxis(
                        ap=labels_sb[:, k:k + 1], axis=0),
                )
                nc.gpsimd.indirect_dma_start(
                    out=bg[:, k:k + 1],
                    out_offset=None,
                    in_=bcol[:, :],
                    in_offset=bass.IndirectOffsetOnAxis(
                        ap=labels_sb[:, k:k + 1], axis=0),
                )
            prod = sp.tile([RPC, KPOS * D], f32)
            x_bc = (xs_sb[:].rearrange("p (o d) -> p o d", o=1)
                    .to_broadcast([RPC, KPOS, D]))
            nc.vector.tensor_tensor(
                out=prod[:].rearrange("p (k d) -> p k d", k=KPOS),
                in0=gat[:].rearrange("p (k d) -> p k d", k=KPOS),
                in1=x_bc,
                op=ALU.mult,
            )
            pos_logits = sp.tile([RPC, KPOS], f32)
            nc.vector.reduce_sum(
                out=pos_logits[:],
                in_=prod[:].rearrange("p (k d) -> p k d", k=KPOS),
                axis=AX.X,
            )
            nc.vector.tensor_add(out=pos_logits[:], in0=pos_logits[:], in1=bg[:])

            # ---- main expsum stream over all classes ----
            for ci in range(NCHUNKS):
                off = ci * NF
                nf = min(NF, CHALF - off)
                wtile = wpool.tile([128, NF], bf16, tag="w")
                nc.sync.dma_start(out=wtile[:, :nf], in_=w2[:, off:off + nf])
                if has_bias:
                    btile = wpool.tile([33, NF], f32, tag="b")
                    nc.sync.dma_start(out=btile[0:1, :nf],
                                      in_=b2_d[0:1, off:off + nf])
                    nc.sync.dma_start(out=btile[32:33, :nf],
                                      in_=b2_d[1:2, off:off + nf])
                ps = pp.tile([128, PSUM_W], f32, tag="ps")
                for mo in range(0, nf, 512):
                    ns = min(512, nf - mo)
                    nc.tensor.matmul(
                        out=ps[:, mo:mo + ns],
                        lhsT=xt_sb[0:64, :],
                        rhs=wtile[0:64, mo:mo + ns],
                        start=True, stop=not has_bias,
                    )
                    nc.tensor.matmul(
                        out=ps[:, NF + mo:NF + mo + ns],
                        lhsT=xt_sb[64:128, :],
                        rhs=wtile[64:128, mo:mo + ns],
                        start=True, stop=not has_bias,
                    )
                    if has_bias:
                        nc.tensor.matmul(
                            out=ps[:, mo:mo + ns],
                            lhsT=ones33[0:1, :],
                            rhs=btile[0:1, mo:mo + ns],
                            start=False, stop=True,
                        )
                        nc.tensor.matmul(
                            out=ps[:, NF + mo:NF + mo + ns],
                            lhsT=ones33[32:33, :],
                            rhs=btile[32:33, mo:mo + ns],
                            start=False, stop=True,
                        )
                ps3 = ps[:].rearrange("p (h n) -> p h n", h=2)[:, :, 0:nf]
                es3 = es[:].rearrange("p (h n) -> p h n", h=2)[:, :, 0:nf]
                nc.scalar.activation(
                    out=es3, in_=ps3, func=AF.Exp,
                    accum_out=acc[:, ci:ci + 1],
                )

            # ---- combine ----
            total = sp.tile([128, 1], f32)
            nc.vector.reduce_sum(out=total[:], in_=acc[:], axis=AX.X)
            pos_e = sp.tile([RPC, KPOS], f32)
            pos_sum = sp.tile([RPC, 1], f32)
            nc.scalar.activation(out=pos_e[:], in_=pos_logits[:], func=AF.Exp,
                                 accum_out=pos_sum[:])
            neg = sp.tile([RPC, 1], f32)
            nc.vector.tensor_sub(out=neg[:], in0=total[:], in1=pos_sum[:])
            denom = sp.tile([RPC, KPOS], f32)
            nc.vector.tensor_tensor(out=denom[:], in0=pos_e[:],
                                    in1=neg[:].to_broadcast([RPC, KPOS]),
                                    op=ALU.add)
            logd = sp.tile([RPC, KPOS], f32)
            nc.scalar.activation(out=logd[:], in_=denom[:], func=AF.Ln)
            losses = sp.tile([RPC, KPOS], f32)
            nc.vector.tensor_sub(out=losses[:], in0=logd[:], in1=pos_logits[:])
            row = sp.tile([RPC, 1], f32)
            nc.vector.reduce_sum(out=row[:], in_=losses[:], axis=AX.X)
            rows = sp.tile([RPC, 1], f32)
            nc.vector.tensor_scalar_mul(out=rows[:], in0=row[:],
                                        scalar1=1.0 / (B * KPOS))
            ones = sp.tile([128, 1], f32)
            nc.vector.memset(ones[:], 1.0)
            ps1 = pp.tile([1, 1], f32, tag="ps")
            nc.tensor.matmul(out=ps1[:], lhsT=ones[:], rhs=rows[:],
                             start=True, stop=True)
            loss_sb = sp.tile([1, 1], f32)
            nc.scalar.copy(out=loss_sb[:], in_=ps1[:])
            nc.sync.dma_start(out=loss_d[:], in_=loss_sb[:])

    nc.compile()
    return nc


def make_in_maps(x, labels, W, b, has_bias):
    import ml_dtypes
    bf = ml_dtypes.bfloat16
    w2 = np.ascontiguousarray(
        np.concatenate([W[:, :CHALF], W[:, CHALF:]], axis=0).astype(bf))
    wt = np.ascontiguousarray(W.T)
    bcol = np.ascontiguousarray(b.reshape(C, 1))
    b2 = np.ascontiguousarray(np.stack([b[:CHALF], b[CHALF:]]))
    in_maps = []
    for c in range(NCORES):
        xs = np.ascontiguousarray(x[c * RPC:(c + 1) * RPC])
        xt = np.ascontiguousarray(xs.T)
        xt2 = np.ascontiguousarray(
            np.concatenate([xt, xt], axis=0).astype(bf))
        lab = np.ascontiguousarray(
            labels[c * RPC:(c + 1) * RPC].astype(np.int32))
        m = {"w2": w2, "xt2": xt2, "wt": wt, "bcol": bcol,
             "labels": lab, "xs": xs}
        if has_bias:
            m["b2"] = b2
        in_maps.append(m)
    return in_maps


_PROGRAM_CACHE = {}


def kernel(x=None, labels=None, W=None, b=None, **_ignored):
    _ensure_concourse()
    from concourse.bass_utils import run_bass_kernel_spmd

    x = np.asarray(x, dtype=np.float32)
    W = np.asarray(W, dtype=np.float32)
    b = np.asarray(b, dtype=np.float32)
    labels = np.asarray(labels)
    has_bias = bool(np.any(b))

    if has_bias not in _PROGRAM_CACHE:
        _PROGRAM_CACHE[has_bias] = build_program(has_bias)
    nc = _PROGRAM_CACHE[has_bias]

    in_maps = make_in_maps(x, labels, W, b, has_bias)
    res = run_bass_kernel_spmd(nc, in_maps, list(range(NCORES))).results
    out = np.float64(0.0)
    for r in res:
        out += np.float64(r["loss"][0, 0])
    return np.float32(out)


# revision 2
# speedup vs baseline: 1.1455x; 1.1455x over previous
"""Trainium2 Bass kernel for nn_ModelWithLoss_67808943669610.

Computes, for the full (unsharded) inputs:
    logits = x @ W + b                       # [B, C]
    total  = sum_c exp(logits)               # per row
    pos    = logits gathered at labels       # [B, K]
    loss   = mean over (B*K) of log(exp(pos) + total - sum_k exp(pos)) - pos

Sharding: data-parallel over the batch. Each of the 8 cores gets 128 rows of
x/labels and a full copy of W (bf16, laid out as two stacked 64-row halves so
DMA tiles span 128 partitions at full bandwidth). Per core:
  - PE streams W through two persistent K=64 weight blocks (xT duplicated in
    rows 0-63 / 64-127, pre-scaled by 1/64) producing logits/64 in PSUM,
    2048 classes per chunk.
  - exp + free-axis sum of each chunk alternates between ScalarE
    (ACTIVATE Exp with scale=64 and accum_out) and VectorE (a custom DVE op
    computing (1 + l/64)^64 by six squarings with fused accumulate) so the
    two transcendental-capable engines run concurrently.
  - Positive logits are computed separately in fp32: indirect-DMA gather of
    the 640 needed W^T rows + a DVE dot against x.
  - Final per-core scalar = sum of per-(row,positive) losses / (B*K); the
    host just sums the 8 per-core scalars.
The max-subtraction in the reference cancels algebraically; logits here are
O(1) so unshifted exp is numerically safe in fp32.
"""

import numpy as np

B, D, C, KPOS = 1024, 64, 100000, 5
NCORES = 8
RPC = B // NCORES          # 128 rows per core
CHALF = C // 2             # 50000 classes per half-block
NF = 1024                  # classes per half-block per PSUM chunk
PSUM_W = 2 * NF            # psum tile free width (two half-blocks)
NCHUNKS = -(-CHALF // NF)  # 49
WTILE = 8192               # w2 columns per DMA tile (16KB/partition in bf16)
SCALE = 64.0               # logits are computed as l/SCALE on-device


def _ensure_concourse():
    try:
        import concourse  # noqa: F401
    except ImportError:
        import sys
        for p in ("/opt/trn_rl_repo", "/root/.axon_site/_ro/trn_rl_repo"):
            if p not in sys.path:
                sys.path.insert(0, p)


_EXPSQ = None


def _register_exp_sq6():
    """Register a custom DVE op: out = (1 + in0)^64, accum_out = row sum.

    With in0 = l/64 this approximates exp(l) to a relative error of
    ~l^2/128 (< 1% for |l| <= 1; the systematic effect on the summed
    denominator is ~2e-4, i.e. ~2e-5 on the final loss).
    """
    global _EXPSQ
    if _EXPSQ is not None:
        return _EXPSQ
    from operator import add as _add
    import concourse.dve_ops as dve_ops
    from concourse.dve_spec import Spec, Src0, One, Zero, sq, lower
    from concourse.dve_uop import DveOpSpec

    name = "EXP_SQ6_ANT"
    for o in dve_ops.OPS:
        if o.name == name:
            _EXPSQ = o
            return o

    body = Src0 + One
    for _ in range(6):
        body = sq(body)

    def _ref(in0, in1, c0, c1, c2):
        u = 1.0 + in0.astype(np.float32)
        out = u
        for _ in range(6):
            out = (out * out).astype(np.float32)
        return out, out.reshape(out.shape[0], -1).sum(axis=-1, keepdims=True)

    spec = Spec(body=body, accum=_add, accum_init=Zero, reference=_ref)
    row = max(dve_ops._SUB_OPCODE_FOR_NAME.values()) + 1
    assert row < 0x20
    dve_ops._SUB_OPCODE_FOR_NAME[name] = row
    shas = {}
    for ver in ("v3", "v4"):
        u = lower(spec, ver=ver)
        shas[ver] = DveOpSpec(name=name, opcode=row, uops=u, rd1_en=False).sha(ver)
    op = dve_ops.DveOp(name, spec, subdim=False, uops_sha=shas)
    dve_ops.OPS.append(op)
    dve_ops.CUSTOM_DVE_SPECS[name] = spec
    _EXPSQ = op
    return op


def _chunk_schedule(has_bias):
    """Greedy ACT/DVE assignment of the 49 exp chunks, balancing engine ns."""
    chunks = []
    for wo in range(0, CHALF, WTILE):
        wcols = min(WTILE, CHALF - wo)
        for so in range(0, wcols, NF):
            chunks.append((wo, so, min(NF, wcols - so)))
    act_cost = 4500.0   # table loads + positives exp/ln tail live on ACT
    dve_cost = 2500.0   # positives dot/reduce + combine live on DVE
    sched = []
    for (_, _, nf) in chunks:
        a = (172 + 2 * nf) / 1.2 + 283
        v = (120 + 2 * nf) / 0.96
        if act_cost + a / 2 <= dve_cost + v / 2:
            sched.append("act")
            act_cost += a
        else:
            sched.append("dve")
            dve_cost += v
    return chunks, sched


def build_program(has_bias: bool):
    _ensure_concourse()
    import concourse.bass as bass
    import concourse.bacc as bacc
    import concourse.mybir as mybir
    import concourse.tile as tile

    expsq = _register_exp_sq6()

    f32 = mybir.dt.float32
    bf16 = mybir.dt.bfloat16
    i32 = mybir.dt.int32
    AF = mybir.ActivationFunctionType
    ALU = mybir.AluOpType
    AX = mybir.AxisListType

    nc = bacc.Bacc(
        "TRN2",
        target_bir_lowering=False,
        debug=False,
        num_devices=NCORES,
    )

    w2 = nc.dram_tensor("w2", [128, CHALF], bf16, kind="ExternalInput")
    xt2 = nc.dram_tensor("xt2", [128, 128], bf16, kind="ExternalInput")
    wt = nc.dram_tensor("wt", [C, D], f32, kind="ExternalInput")
    bcol = nc.dram_tensor("bcol", [C, 1], f32, kind="ExternalInput")
    labels_d = nc.dram_tensor("labels", [RPC, KPOS], i32, kind="ExternalInput")
    xs_d = nc.dram_tensor("xs", [RPC, D], f32, kind="ExternalInput")
    if has_bias:
        b2_d = nc.dram_tensor("b2", [2, CHALF], f32, kind="ExternalInput")
    loss_d = nc.dram_tensor("loss", [1, 1], f32, kind="ExternalOutput")

    chunks, sched = _chunk_schedule(has_bias)

    with tile.TileContext(nc) as tc:
        with (
            tc.tile_pool(name="wpool", bufs=3) as wpool,
            tc.tile_pool(name="psum", bufs=2, space="PSUM") as pp,
            tc.tile_pool(name="esp", bufs=1) as esp,
            tc.tile_pool(name="small", bufs=1) as sp,
        ):
            xt_sb = sp.tile([128, 128], bf16)
            nc.sync.dma_start(out=xt_sb[:], in_=xt2[:])
            acc = sp.tile([128, NCHUNKS], f32)
            es = esp.tile([128, PSUM_W], bf16)    # ACT exp out (discarded)
            ev = esp.tile([128, PSUM_W], bf16)    # DVE exp out (discarded)

            if has_bias:
                ones33 = sp.tile([33, 128], f32)
                nc.vector.memset(ones33[:], 1.0)

            # ---- positives: gather the needed W^T rows, dot with x ----
            labels_sb = sp.tile([RPC, KPOS], i32)
            nc.sync.dma_start(out=labels_sb[:], in_=labels_d[:])
            xs_sb = sp.tile([RPC, D], f32)
            nc.sync.dma_start(out=xs_sb[:], in_=xs_d[:])
            gat = sp.tile([RPC, KPOS * D], f32)
            bg = sp.tile([RPC, KPOS], f32)
            for k in range(KPOS):
                nc.gpsimd.indirect_dma_start(
                    out=gat[:, k * D:(k + 1) * D],
                    out_offset=None,
                    in_=wt[:, :],
                    in_offset=bass.IndirectOffsetOnAxis(
                        ap=labels_sb[:, k:k + 1], axis=0),
                )
                nc.gpsimd.indirect_dma_start(
                    out=bg[:, k:k + 1],
                    out_offset=None,
                    in_=bcol[:, :],
                    in_offset=bass.IndirectOffsetOnAxis(
                        ap=labels_sb[:, k:k + 1], axis=0),
                )
            prod = sp.tile([RPC, KPOS * D], f32)
            x_bc = (xs_sb[:].rearrange("p (o d) -> p o d", o=1)
                    .to_broadcast([RPC, KPOS, D]))
            nc.vector.tensor_tensor(
                out=prod[:].rearrange("p (k d) -> p k d", k=KPOS),
                in0=gat[:].rearrange("p (k d) -> p k d", k=KPOS),
                in1=x_bc,
                op=ALU.mult,
            )
            pos_logits = sp.tile([RPC, KPOS], f32)
            nc.vector.reduce_sum(
                out=pos_logits[:],
                in_=prod[:].rearrange("p (k d) -> p k d", k=KPOS),
                axis=AX.X,
            )
            nc.vector.tensor_add(out=pos_logits[:], in0=pos_logits[:], in1=bg[:])

            # ---- main expsum stream over all classes ----
            ci = 0
            for wo in range(0, CHALF, WTILE):
                wcols = min(WTILE, CHALF - wo)
                wtile = wpool.tile([128, WTILE], bf16, tag="w")
                nc.sync.dma_start(out=wtile[:, :wcols], in_=w2[:, wo:wo + wcols])
                if has_bias:
                    btile = wpool.tile([33, WTILE], f32, tag="b")
                    nc.sync.dma_start(out=btile[0:1, :wcols],
                                      in_=b2_d[0:1, wo:wo + wcols])
                    nc.sync.dma_start(out=btile[32:33, :wcols],
                                      in_=b2_d[1:2, wo:wo + wcols])
                for so in range(0, wcols, NF):
                    nf = min(NF, wcols - so)
                    ps = pp.tile([128, PSUM_W], f32, tag="ps")
                    for mo in range(0, nf, 512):
                        ns = min(512, nf - mo)
                        nc.tensor.matmul(
                            out=ps[:, mo:mo + ns],
                            lhsT=xt_sb[0:64, :],
                            rhs=wtile[0:64, so + mo:so + mo + ns],
                            start=True, stop=not has_bias,
                        )
                        nc.tensor.matmul(
                            out=ps[:, NF + mo:NF + mo + ns],
                            lhsT=xt_sb[64:128, :],
                            rhs=wtile[64:128, so + mo:so + mo + ns],
                            start=True, stop=not has_bias,
                        )
                        if has_bias:
                            nc.tensor.matmul(
                                out=ps[:, mo:mo + ns],
                                lhsT=ones33[0:1, :],
                                rhs=btile[0:1, so + mo:so + mo + ns],
                                start=False, stop=True,
                            )
                            nc.tensor.matmul(
                                out=ps[:, NF + mo:NF + mo + ns],
                                lhsT=ones33[32:33, :],
                                rhs=btile[32:33, so + mo:so + mo + ns],
                                start=False, stop=True,
                            )
                    accw = acc[:, ci:ci + 1]
                    if sched[ci] == "act":
                        ps3 = ps[:].rearrange("p (h n) -> p h n", h=2)[:, :, 0:nf]
                        es3 = es[:].rearrange("p (h n) -> p h n", h=2)[:, :, 0:nf]
                        nc.scalar.activation(out=es3, in_=ps3, func=AF.Exp,
                                             scale=float(SCALE), accum_out=accw)
                    else:
                        if nf == NF:
                            in0 = ps[:, 0:PSUM_W]
                            out0 = ev[:, 0:PSUM_W]
                        else:
                            in0 = ps[:].rearrange("p (h n) -> p h n", h=2)[:, :, 0:nf]
                            out0 = ev[:].rearrange("p (h n) -> p h n", h=2)[:, :, 0:nf]
                        nc.vector._custom_dve(expsq, out=out0, in0=in0,
                                              accum_out=accw)
                    ci += 1
            assert ci == NCHUNKS

            # ---- combine ----
            total = sp.tile([128, 1], f32)
            nc.vector.reduce_sum(out=total[:], in_=acc[:], axis=AX.X)
            pos_e = sp.tile([RPC, KPOS], f32)
            pos_sum = sp.tile([RPC, 1], f32)
            nc.scalar.activation(out=pos_e[:], in_=pos_logits[:], func=AF.Exp,
                                 accum_out=pos_sum[:])
            neg = sp.tile([RPC, 1], f32)
            nc.vector.tensor_sub(out=neg[:], in0=total[:], in1=pos_sum[:])
            denom = sp.tile([RPC, KPOS], f32)
            nc.vector.tensor_tensor(out=denom[:], in0=pos_e[:],
                                    in1=neg[:].to_broadcast([RPC, KPOS]),
                                    op=ALU.add)
            logd = sp.tile([RPC, KPOS], f32)
            nc.scalar.activation(out=logd[:], in_=denom[:], func=AF.Ln)
            losses = sp.tile([RPC, KPOS], f32)
            nc.vector.tensor_sub(out=losses[:], in0=logd[:], in1=pos_logits[:])
            row = sp.tile([RPC, 1], f32)
            nc.vector.reduce_sum(out=row[:], in_=losses[:], axis=AX.X)
            rows = sp.tile([RPC, 1], f32)
            nc.vector.tensor_scalar_mul(out=rows[:], in0=row[:],
                                        scalar1=1.0 / (B * KPOS))
            ones = sp.tile([128, 1], f32)
            nc.vector.memset(ones[:], 1.0)
            ps1 = pp.tile([1, 1], f32, tag="ps")
            nc.tensor.matmul(out=ps1[:], lhsT=ones[:], rhs=rows[:],
                             start=True, stop=True)
            loss_sb = sp.tile([1, 1], f32)
            nc.scalar.copy(out=loss_sb[:], in_=ps1[:])
            nc.sync.dma_start(out=loss_d[:], in_=loss_sb[:])

    nc.compile()
    return nc


def make_in_maps(x, labels, W, b, has_bias):
    import ml_dtypes
    bf = ml_dtypes.bfloat16
    w2 = np.ascontiguousarray(
        np.concatenate([W[:, :CHALF], W[:, CHALF:]], axis=0).astype(bf))
    wt = np.ascontiguousarray(W.T)
    bcol = np.ascontiguousarray(b.reshape(C, 1))
    b2 = np.ascontiguousarray(np.stack([b[:CHALF], b[CHALF:]]) / SCALE)
    in_maps = []
    for c in range(NCORES):
        xs = np.ascontiguousarray(x[c * RPC:(c + 1) * RPC])
        xt = np.ascontiguousarray(xs.T) / SCALE
        xt2 = np.ascontiguousarray(
            np.concatenate([xt, xt], axis=0).astype(bf))
        lab = np.ascontiguousarray(
            labels[c * RPC:(c + 1) * RPC].astype(np.int32))
        m = {"w2": w2, "xt2": xt2, "wt": wt, "bcol": bcol,
             "labels": lab, "xs": xs}
        if has_bias:
            m["b2"] = b2
        in_maps.append(m)
    return in_maps


_PROGRAM_CACHE = {}


def kernel(x=None, labels=None, W=None, b=None, **_ignored):
    _ensure_concourse()
    from concourse.bass_utils import run_bass_kernel_spmd

    x = np.asarray(x, dtype=np.float32)
    W = np.asarray(W, dtype=np.float32)
    b = np.asarray(b, dtype=np.float32)
    labels = np.asarray(labels)
    has_bias = bool(np.any(b))

    if has_bias not in _PROGRAM_CACHE:
        _PROGRAM_CACHE[has_bias] = build_program(has_bias)
    nc = _PROGRAM_CACHE[has_bias]

    in_maps = make_in_maps(x, labels, W, b, has_bias)
    res = run_bass_kernel_spmd(nc, in_maps, list(range(NCORES))).results
    out = np.float64(0.0)
    for r in res:
        out += np.float64(r["loss"][0, 0])
    return np.float32(out)


# revision 6
# speedup vs baseline: 1.3375x; 1.1675x over previous
"""Trainium2 Bass kernel for nn_ModelWithLoss_67808943669610.

Computes, for the full (unsharded) inputs:
    logits = x @ W + b                       # [B, C]
    total  = sum_c exp(logits)               # per row
    pos    = logits gathered at labels       # [B, K]
    loss   = mean over (B*K) of log(exp(pos) + total - sum_k exp(pos)) - pos

Sharding: data-parallel over the batch. Each of the 8 cores gets 128 rows of
x/labels and a full copy of W (bf16, laid out as two stacked 64-row halves so
DMA tiles span 128 partitions at full bandwidth). Per core:
  - PE streams W through two persistent K=64 weight blocks (xT duplicated in
    rows 0-63 / 64-127, pre-scaled by 1/64) producing logits/64 in PSUM,
    2048 classes per chunk.
  - exp + free-axis sum of each chunk alternates between ScalarE
    (ACTIVATE Exp with scale=64 and accum_out) and VectorE (a custom DVE op
    computing (1 + l/64)^64 by six squarings with fused accumulate) so the
    two transcendental-capable engines run concurrently.
  - Positive logits are computed separately in fp32: indirect-DMA gather of
    the 640 needed W^T rows + a DVE dot against x.
  - Final per-core scalar = sum of per-(row,positive) losses / (B*K); the
    host just sums the 8 per-core scalars.
The max-subtraction in the reference cancels algebraically; logits here are
O(1) so unshifted exp is numerically safe in fp32.
"""

import numpy as np

B, D, C, KPOS = 1024, 64, 100000, 5
NCORES = 8
RPC = B // NCORES          # 128 rows per core
CHALF = C // 2             # 50000 classes per half-block
NF = 512                   # classes per half-block per PSUM chunk (1 bank)
PSUM_W = 2 * NF            # psum tile free width (two half-blocks, 2 banks)
NCHUNKS = -(-CHALF // NF)  # 98
WTILES = [2048, 8192, 8192, 8192, 8192, 8192, 6992]  # w2 cols per DMA tile
SCALE = 64.0               # logits are computed as l/SCALE on-device


def _ensure_concourse():
    try:
        import concourse  # noqa: F401
    except ImportError:
        import sys
        for p in ("/opt/trn_rl_repo", "/root/.axon_site/_ro/trn_rl_repo"):
            if p not in sys.path:
                sys.path.insert(0, p)


_EXPSQ = None


def _register_exp_sq6():
    """Register a custom DVE op: out = (1 + in0)^64, accum_out = row sum.

    With in0 = l/64 this approximates exp(l) to a relative error of
    ~l^2/128 (< 1% for |l| <= 1; the systematic effect on the summed
    denominator is ~2e-4, i.e. ~2e-5 on the final loss).
    """
    global _EXPSQ
    if _EXPSQ is not None:
        return _EXPSQ
    from operator import add as _add
    import concourse.dve_ops as dve_ops
    from concourse.dve_spec import Spec, Src0, One, Zero, sq, lower
    from concourse.dve_uop import DveOpSpec

    name = "EXP_SQ6_ANT"
    for o in dve_ops.OPS:
        if o.name == name:
            _EXPSQ = o
            return o

    body = Src0 + One
    for _ in range(6):
        body = sq(body)

    def _ref(in0, in1, c0, c1, c2):
        u = 1.0 + in0.astype(np.float32)
        out = u
        for _ in range(6):
            out = (out * out).astype(np.float32)
        return out, out.reshape(out.shape[0], -1).sum(axis=-1, keepdims=True)

    spec = Spec(body=body, accum=_add, accum_init=Zero, reference=_ref)
    row = max(dve_ops._SUB_OPCODE_FOR_NAME.values()) + 1
    assert row < 0x20
    dve_ops._SUB_OPCODE_FOR_NAME[name] = row
    shas = {}
    for ver in ("v3", "v4"):
        u = lower(spec, ver=ver)
        shas[ver] = DveOpSpec(name=name, opcode=row, uops=u, rd1_en=False).sha(ver)
    op = dve_ops.DveOp(name, spec, subdim=False, uops_sha=shas)
    dve_ops.OPS.append(op)
    dve_ops.CUSTOM_DVE_SPECS[name] = spec
    _EXPSQ = op
    return op


def _chunk_schedule(has_bias):
    """Greedy ACT/DVE assignment of the exp chunks, balancing engine ns."""
    assert sum(WTILES) == CHALF
    chunks = []
    wo = 0
    for wcols in WTILES:
        for so in range(0, wcols, NF):
            chunks.append((wo, so, min(NF, wcols - so)))
        wo += wcols
    act_cost = 4500.0   # table loads + positives exp/ln tail live on ACT
    dve_cost = 2500.0   # positives dot/reduce + combine live on DVE
    sched = []
    for (_, _, nf) in chunks:
        a = (172 + 2 * nf) / 1.2 + 283
        v = (120 + 2 * nf) / 0.96 + 84
        if act_cost + a / 2 <= dve_cost + v / 2:
            sched.append("act")
            act_cost += a
        else:
            sched.append("dve")
            dve_cost += v
    return chunks, sched


def build_program(has_bias: bool):
    _ensure_concourse()
    import concourse.bass as bass
    import concourse.bacc as bacc
    import concourse.mybir as mybir
    import concourse.tile as tile

    expsq = _register_exp_sq6()

    f32 = mybir.dt.float32
    bf16 = mybir.dt.bfloat16
    i32 = mybir.dt.int32
    AF = mybir.ActivationFunctionType
    ALU = mybir.AluOpType
    AX = mybir.AxisListType

    nc = bacc.Bacc(
        "TRN2",
        target_bir_lowering=False,
        debug=False,
        num_devices=NCORES,
    )

    w2 = nc.dram_tensor("w2", [128, CHALF], bf16, kind="ExternalInput")
    xt2 = nc.dram_tensor("xt2", [128, 128], bf16, kind="ExternalInput")
    wt = nc.dram_tensor("wt", [C, D], f32, kind="ExternalInput")
    bcol = nc.dram_tensor("bcol", [C, 1], f32, kind="ExternalInput")
    labels_d = nc.dram_tensor("labels", [RPC, KPOS], i32, kind="ExternalInput")
    xs_d = nc.dram_tensor("xs", [RPC, D], f32, kind="ExternalInput")
    if has_bias:
        b2_d = nc.dram_tensor("b2", [2, CHALF], f32, kind="ExternalInput")
    loss_d = nc.dram_tensor("loss", [1, 1], f32, kind="ExternalOutput")

    chunks, sched = _chunk_schedule(has_bias)

    with tile.TileContext(nc) as tc:
        with (
            tc.tile_pool(name="wpool", bufs=3) as wpool,
            tc.tile_pool(name="psum", bufs=4, space="PSUM") as pp,
            tc.tile_pool(name="esp", bufs=1) as esp,
            tc.tile_pool(name="small", bufs=1) as sp,
        ):
            xt_sb = sp.tile([128, 128], bf16)
            nc.sync.dma_start(out=xt_sb[:], in_=xt2[:])
            acc = sp.tile([128, NCHUNKS], f32)
            es = esp.tile([128, PSUM_W], bf16)    # ACT exp out (discarded)
            ev = esp.tile([128, PSUM_W], bf16)    # DVE exp out (discarded)

            if has_bias:
                ones33 = sp.tile([33, 128], f32)
                nc.vector.memset(ones33[:], 1.0)

            # ---- positives: gather the needed W^T rows, dot with x ----
            labels_sb = sp.tile([RPC, KPOS], i32)
            nc.sync.dma_start(out=labels_sb[:], in_=labels_d[:])
            xs_sb = sp.tile([RPC, D], f32)
            nc.sync.dma_start(out=xs_sb[:], in_=xs_d[:])
            gat = sp.tile([RPC, KPOS * D], f32)
            bg = sp.tile([RPC, KPOS], f32)
            for k in range(KPOS):
                nc.gpsimd.indirect_dma_start(
                    out=gat[:, k * D:(k + 1) * D],
                    out_offset=None,
                    in_=wt[:, :],
                    in_offset=bass.IndirectOffsetOnAxis(
                        ap=labels_sb[:, k:k + 1], axis=0),
                )
                nc.gpsimd.indirect_dma_start(
                    out=bg[:, k:k + 1],
                    out_offset=None,
                    in_=bcol[:, :],
                    in_offset=bass.IndirectOffsetOnAxis(
                        ap=labels_sb[:, k:k + 1], axis=0),
                )
            prod = sp.tile([RPC, KPOS * D], f32)
            x_bc = (xs_sb[:].rearrange("p (o d) -> p o d", o=1)
                    .to_broadcast([RPC, KPOS, D]))
            nc.vector.tensor_tensor(
                out=prod[:].rearrange("p (k d) -> p k d", k=KPOS),
                in0=gat[:].rearrange("p (k d) -> p k d", k=KPOS),
                in1=x_bc,
                op=ALU.mult,
            )
            pos_logits = sp.tile([RPC, KPOS], f32)
            nc.vector.reduce_sum(
                out=pos_logits[:],
                in_=prod[:].rearrange("p (k d) -> p k d", k=KPOS),
                axis=AX.X,
            )
            nc.vector.tensor_add(out=pos_logits[:], in0=pos_logits[:], in1=bg[:])

            # ---- main expsum stream over all classes ----
            ci = 0
            wo = 0
            for wcols in WTILES:
                wtile = wpool.tile([128, max(WTILES)], bf16, tag="w")
                nc.sync.dma_start(out=wtile[:, :wcols], in_=w2[:, wo:wo + wcols])
                if has_bias:
                    btile = wpool.tile([33, max(WTILES)], f32, tag="b")
                    nc.sync.dma_start(out=btile[0:1, :wcols],
                                      in_=b2_d[0:1, wo:wo + wcols])
                    nc.sync.dma_start(out=btile[32:33, :wcols],
                                      in_=b2_d[1:2, wo:wo + wcols])
                for so in range(0, wcols, NF):
                    ns = min(NF, wcols - so)
                    ps = pp.tile([128, PSUM_W], f32, tag="ps")
                    nc.tensor.matmul(
                        out=ps[:, 0:ns],
                        lhsT=xt_sb[0:64, :],
                        rhs=wtile[0:64, so:so + ns],
                        start=True, stop=not has_bias,
                    )
                    nc.tensor.matmul(
                        out=ps[:, NF:NF + ns],
                        lhsT=xt_sb[64:128, :],
                        rhs=wtile[64:128, so:so + ns],
                        start=True, stop=not has_bias,
                    )
                    if has_bias:
                        nc.tensor.matmul(
                            out=ps[:, 0:ns],
                            lhsT=ones33[0:1, :],
                            rhs=btile[0:1, so:so + ns],
                            start=False, stop=True,
                        )
                        nc.tensor.matmul(
                            out=ps[:, NF:NF + ns],
                            lhsT=ones33[32:33, :],
                            rhs=btile[32:33, so:so + ns],
                            start=False, stop=True,
                        )
                    accw = acc[:, ci:ci + 1]
                    if sched[ci] == "act":
                        if ns == NF:
                            in0 = ps[:, 0:PSUM_W]
                            out0 = es[:, 0:PSUM_W]
                        else:
                            in0 = ps[:].rearrange("p (h n) -> p h n", h=2)[:, :, 0:ns]
                            out0 = es[:].rearrange("p (h n) -> p h n", h=2)[:, :, 0:ns]
                        nc.scalar.activation(out=out0, in_=in0, func=AF.Exp,
                                             scale=float(SCALE), accum_out=accw)
                    else:
                        if ns == NF:
                            in0 = ps[:, 0:PSUM_W]
                            out0 = ev[:, 0:PSUM_W]
                        else:
                            in0 = ps[:].rearrange("p (h n) -> p h n", h=2)[:, :, 0:ns]
                            out0 = ev[:].rearrange("p (h n) -> p h n", h=2)[:, :, 0:ns]
                        nc.vector._custom_dve(expsq, out=out0, in0=in0,
                                              accum_out=accw)
                    ci += 1
                wo += wcols
            assert ci == NCHUNKS

            # ---- combine ----
            total = sp.tile([128, 1], f32)
            nc.vector.reduce_sum(out=total[:], in_=acc[:], axis=AX.X)
            pos_e = sp.tile([RPC, KPOS], f32)
            pos_sum = sp.tile([RPC, 1], f32)
            nc.scalar.activation(out=pos_e[:], in_=pos_logits[:], func=AF.Exp,
                                 accum_out=pos_sum[:])
            neg = sp.tile([RPC, 1], f32)
            nc.vector.tensor_sub(out=neg[:], in0=total[:], in1=pos_sum[:])
            denom = sp.tile([RPC, KPOS], f32)
            nc.vector.tensor_tensor(out=denom[:], in0=pos_e[:],
                                    in1=neg[:].to_broadcast([RPC, KPOS]),
                                    op=ALU.add)
            logd = sp.tile([RPC, KPOS], f32)
            nc.scalar.activation(out=logd[:], in_=denom[:], func=AF.Ln)
            losses = sp.tile([RPC, KPOS], f32)
            nc.vector.tensor_sub(out=losses[:], in0=logd[:], in1=pos_logits[:])
            row = sp.tile([RPC, 1], f32)
            nc.vector.reduce_sum(out=row[:], in_=losses[:], axis=AX.X)
            rows = sp.tile([RPC, 1], f32)
            nc.vector.tensor_scalar_mul(out=rows[:], in0=row[:],
                                        scalar1=1.0 / (B * KPOS))
            ones = sp.tile([128, 1], f32)
            nc.vector.memset(ones[:], 1.0)
            ps1 = pp.tile([1, 1], f32, tag="ps")
            nc.tensor.matmul(out=ps1[:], lhsT=ones[:], rhs=rows[:],
                             start=True, stop=True)
            loss_sb = sp.tile([1, 1], f32)
            nc.scalar.copy(out=loss_sb[:], in_=ps1[:])
            nc.sync.dma_start(out=loss_d[:], in_=loss_sb[:])

    nc.compile()
    return nc


def make_in_maps(x, labels, W, b, has_bias):
    import ml_dtypes
    bf = ml_dtypes.bfloat16
    w2 = np.ascontiguousarray(
        np.concatenate([W[:, :CHALF], W[:, CHALF:]], axis=0).astype(bf))
    wt = np.ascontiguousarray(W.T)
    bcol = np.ascontiguousarray(b.reshape(C, 1))
    b2 = np.ascontiguousarray(np.stack([b[:CHALF], b[CHALF:]]) / SCALE)
    in_maps = []
    for c in range(NCORES):
        xs = np.ascontiguousarray(x[c * RPC:(c + 1) * RPC])
        xt = np.ascontiguousarray(xs.T) / SCALE
        xt2 = np.ascontiguousarray(
            np.concatenate([xt, xt], axis=0).astype(bf))
        lab = np.ascontiguousarray(
            labels[c * RPC:(c + 1) * RPC].astype(np.int32))
        m = {"w2": w2, "xt2": xt2, "wt": wt, "bcol": bcol,
             "labels": lab, "xs": xs}
        if has_bias:
            m["b2"] = b2
        in_maps.append(m)
    return in_maps


_PROGRAM_CACHE = {}


def kernel(x=None, labels=None, W=None, b=None, **_ignored):
    _ensure_concourse()
    from concourse.bass_utils import run_bass_kernel_spmd

    x = np.asarray(x, dtype=np.float32)
    W = np.asarray(W, dtype=np.float32)
    b = np.asarray(b, dtype=np.float32)
    labels = np.asarray(labels)
    has_bias = bool(np.any(b))

    if has_bias not in _PROGRAM_CACHE:
        _PROGRAM_CACHE[has_bias] = build_program(has_bias)
    nc = _PROGRAM_CACHE[has_bias]

    in_maps = make_in_maps(x, labels, W, b, has_bias)
    res = run_bass_kernel_spmd(nc, in_maps, list(range(NCORES))).results
    out = np.float64(0.0)
    for r in res:
        out += np.float64(r["loss"][0, 0])
    return np.float32(out)
